# revision 1
# baseline (speedup 1.0000x reference)
"""Bass/Tile TRN2 kernel for nn_LzScaleDotAttention (B=8, L=2048, D=512).

Math per batch b:
    S[q,k]   = sum_d Q[q,d] K[k,d]
    E        = exp(S)                       # inputs are pre-scaled small, no max-sub needed
    num[k,d] = sum_q E[q,k] V[q,d]          # = E^T @ V
    den[k]   = sum_q E[q,k]
    mask[k]  = 1.0 if any(V[k,:] != 0) else 0.0
    out[k,d] = num[k,d] * mask[k]*c / (den[k]*mask[k]*c + EPS),  c = 1/sqrt(D)

The renormalisation over the query axis commutes with the E^T@V contraction
(the divisor depends only on k), so the normalised attention matrix is never
materialised: one flash-style pass over q tiles accumulates num (PSUM) and
den (SBUF f32 accumulator + a tiny cross-partition matmul against ones).

Sharding: batch dim (8) across the 8 NeuronCores, one batch per core (SPMD,
no collectives). Matmuls run in float32r (fp32 storage, ~1 cycle/row on the
PE for N=512). Q and K are laid out feature-major ([D, L]) host-side when
sharding, so the device spends no PE cycles transposing operands.
"""

import math
import os
import sys

import numpy as np

for _p in ("/opt/trn_rl_repo", "/root/.axon_site/_ro/trn_rl_repo"):
    if os.path.isdir(_p) and _p not in sys.path:
        sys.path.append(_p)

import concourse.bacc as bacc
import concourse.mybir as mybir
import concourse.tile as tile
from concourse.bass import ds, ts
from concourse.bass_utils import run_bass_kernel_spmd

B, L, D = 8, 2048, 512
P = 128
EPS = 1e-7
N_CORES = 8

f32 = mybir.dt.float32
f32r = mybir.dt.float32r
bf16 = mybir.dt.bfloat16
AF = mybir.ActivationFunctionType
ALU = mybir.AluOpType


def build_program(Lb=L, Db=D, n_cores=N_CORES):
    """Device program. Inputs: qT, kT feature-major [D, L]; v natural [L, D]."""
    NT = Lb // P          # 128-row tiles along q / k timesteps
    DC = Db // P          # 128-wide chunks of the feature dim
    KBW = 512             # k-block width (one PSUM bank of fp32)
    KB = Lb // KBW        # k blocks
    KT = KBW // P         # 128-wide k tiles per block
    QC = Lb // KBW        # 512-wide column chunks of qT
    C = 1.0 / math.sqrt(Db)

    nc = bacc.Bacc(
        "TRN2", target_bir_lowering=False, debug=False, num_devices=n_cores
    )
    qT = nc.dram_tensor("qT", [Db, Lb], bf16, kind="ExternalInput").ap()
    kT = nc.dram_tensor("kT", [Db, Lb], bf16, kind="ExternalInput").ap()
    v = nc.dram_tensor("v", [Lb, Db], f32r, kind="ExternalInput").ap()
    out = nc.dram_tensor("out", [Lb, Db], f32, kind="ExternalOutput").ap()

    with tile.TileContext(nc) as tc:
        with (
            tc.tile_pool(name="const", bufs=1) as cpool,
            tc.tile_pool(name="qTp", bufs=1) as qT_pool,
            tc.tile_pool(name="kTp", bufs=1) as kT_pool,
            tc.tile_pool(name="vSp", bufs=NT) as vS_pool,
            tc.tile_pool(name="warm", bufs=1) as warm_pool,
            tc.tile_pool(name="ep", bufs=6) as e_pool,
            tc.tile_pool(name="accp", bufs=3) as acc_pool,
            tc.tile_pool(name="outp", bufs=4) as out_pool,
            tc.tile_pool(name="scp", bufs=6) as sc_pool,
            tc.tile_pool(name="ps_s", bufs=3, space="PSUM") as ps_s,
            tc.tile_pool(name="ps_num", bufs=1, space="PSUM") as ps_num,
            tc.tile_pool(name="ps_tp", bufs=1, space="PSUM") as ps_tp,
        ):
            ones = cpool.tile([P, 1], f32, name="ones")
            nc.vector.memset(ones, 1.0)
            vmask = cpool.tile([P, NT], f32, name="vmask")

            # PE warm-up: ~4us of dummy fp32 matmuls flips the HAM clock gate
            # to full rate before real work arrives (fp32: 4 cycles/row, so a
            # handful of instructions covers the activity window)
            zf = warm_pool.tile([P, KBW], f32, name="zf")
            nc.vector.memset(zf, 0.0)
            wps = ps_tp.tile([P, KBW], f32, tag="tp", name="wps")
            for w in range(6):
                # all into one psum tile: pure WAW chain, no pool churn
                nc.tensor.matmul(wps, zf[:, :P], zf, start=True, stop=True)

            # Persistent SBUF residents, loaded straight from DRAM.
            # q/k column-chunk tiles [128, 512]: 2KB rows, good DMA shape.
            # kT loads issue on Sync's HWDGE ring, qT on ACT's ring, v on the
            # gpsimd SWDGE ring (casting f32 -> f32r) — three rings in parallel.
            # Each DMA ring sustains only ~120 GB/s, so tiles are assigned to
            # the three rings (Sync-HWDGE, ACT-HWDGE, gpsimd-SWDGE) in the
            # order the flash loop consumes them: k block 0 first, all of q
            # split across two rings (it gates every q-tile of k-block 0),
            # later k blocks last.
            qTs = {}
            kTs = {}

            def load_k(dc, c, eng):
                t_ = kT_pool.tile([P, KBW], bf16, tag=f"kT{dc}_{c}", name=f"kT{dc}_{c}")
                eng.dma_start(t_, kT[ds(dc * P, P), ds(c * KBW, KBW)])
                kTs[(dc, c)] = t_

            def load_q(dc, c, eng):
                t_ = qT_pool.tile([P, KBW], bf16, tag=f"qT{dc}_{c}", name=f"qT{dc}_{c}")
                eng.dma_start(t_, qT[ds(dc * P, P), ds(c * KBW, KBW)])
                qTs[(dc, c)] = t_

            vS_t = [None] * NT

            def load_v(t, eng):
                vt = vS_pool.tile([P, Db], f32r, tag="vS", name=f"vS{t}")
                eng.dma_start(vt, v[ts(t, P), :])
                vS_t[t] = vt
                nc.vector.tensor_reduce(
                    vmask[:, t : t + 1],
                    vt,
                    axis=mybir.AxisListType.X,
                    op=ALU.max,
                    apply_absolute_value=True,
                )

            # Both HWDGE engines share one physical ring (~230 GB/s) whose
            # first transfer lands only after the sync engine's ~8us
            # preamble. The gpsimd SWDGE ring (~100 GB/s) clears its
            # preamble at ~2us, so it bootstraps k block 0 and the first v
            # tiles; the HWDGE ring leads with q (which gates every q-tile
            # of k-block 0), then k block 1, the v tail, k blocks 2-3.
            v_head = min(8, NT)
            for dc in range(DC):
                load_k(dc, 0, nc.gpsimd)
            for c in range(QC):
                for dc in range(DC):
                    load_q(dc, c, nc.sync)
            if KB > 1:
                for dc in range(DC):
                    load_k(dc, 1, nc.sync)
            for t in range(v_head, NT):
                load_v(t, nc.sync)
            for c in range(2, KB):
                for dc in range(DC):
                    load_k(dc, c, nc.sync)
            for t in range(v_head):
                load_v(t, nc.gpsimd)
            # mask[k] = (max_d |v[k,d]|) > 0 -> {0.0, 1.0}; pm = mask * c
            nc.vector.tensor_scalar(vmask, vmask, 0.0, None, op0=ALU.is_gt)
            pm = cpool.tile([P, NT], f32, name="pm")
            nc.vector.tensor_scalar_mul(pm, vmask, C)

            def q_lhsT(qt, dc):
                return qTs[(dc, qt // KT)][:, ts(qt % KT, P)]

            # ---- Main flash loop over k blocks ----
            # The per-block epilogue (den, scale, writeback) is emitted inside
            # the NEXT block's first q-tile so its engine work interleaves
            # with the pipeline refill instead of stalling the PE on PSUM
            # slot reuse at every block boundary.
            def make_epilogue(kb, acc, nums):
                def emit():
                    for kt in range(KT):
                        j = kb * KT + kt
                        dps = ps_tp.tile([P, 1], f32, tag="tp", name=f"dps{j}")
                        nc.tensor.matmul(
                            dps, acc[:, ts(kt, P)], ones, start=True, stop=True
                        )
                        # scale = pm / (den * pm + EPS), pm = mask/sqrt(D)
                        scl = sc_pool.tile([P, 1], f32, tag="scl", name=f"scl{j}")
                        nc.vector.tensor_scalar(
                            scl, dps, pm[:, j : j + 1], EPS,
                            op0=ALU.mult, op1=ALU.add,
                        )
                        rcp = sc_pool.tile([P, 1], f32, tag="rcp", name=f"rcp{j}")
                        nc.vector.reciprocal(rcp, scl)
                        nc.vector.tensor_mul(rcp, rcp, pm[:, j : j + 1])
                        o = out_pool.tile([P, Db], f32, tag="o", name=f"o{j}")
                        nc.vector.tensor_scalar_mul(o, nums[kt], rcp)
                        nc.sync.dma_start(out[ts(j, P), :], o)
                return emit

            pending_epilogue = None
            for kb in range(KB):
                acc = acc_pool.tile([P, KBW], f32, tag="acc", name=f"acc{kb}")
                nums = None
                e_tiles = {}
                # software pipeline: stage-1 (scores+exp) runs one q-tile
                # ahead of stage-2 (E^T @ V) so the PE never waits on ACT
                for qt in range(NT + 1):
                    if qt < NT:
                        s_ps = ps_s.tile([P, KBW], f32, tag="s", name=f"s{kb}_{qt}")
                        for dc in range(DC):
                            nc.tensor.matmul(
                                s_ps,
                                q_lhsT(qt, dc),
                                kTs[(dc, kb)],
                                start=(dc == 0),
                                stop=(dc == DC - 1),
                            )
                        e = e_pool.tile([P, KBW], f32r, tag="e", name=f"e{kb}_{qt}")
                        nc.scalar.activation(e, s_ps, AF.Exp)
                        if qt == 0 and pending_epilogue is not None:
                            # previous block's den/scale/writeback lands here,
                            # after this block's first scores+exp are queued
                            pending_epilogue()
                            pending_epilogue = None
                        if qt == 0:
                            nc.vector.tensor_copy(acc, e)
                        else:
                            nc.vector.tensor_add(acc, acc, e)
                        e_tiles[qt] = e
                    if qt >= 1:
                        if nums is None:
                            # allocate after the previous block's release ops
                            # so the pool trace sees release before alloc
                            nums = [
                                ps_num.tile(
                                    [P, Db], f32,
                                    tag=f"num{kt}", name=f"num{kb}_{kt}",
                                )
                                for kt in range(KT)
                            ]
                        ep = e_tiles.pop(qt - 1)
                        for kt in range(KT):
                            nc.tensor.matmul(
                                nums[kt],
                                ep[:, ts(kt, P)],
                                vS_t[qt - 1],
                                start=(qt - 1 == 0),
                                stop=(qt - 1 == NT - 1),
                            )
                pending_epilogue = make_epilogue(kb, acc, nums)
            pending_epilogue()

    return nc


_cache = {}


def _get_compiled(Lb=L, Db=D):
    key = (Lb, Db)
    if key not in _cache:
        nc = build_program(Lb, Db)
        nc.compile()
        _cache[key] = nc
    return _cache[key]


def run(q, k, v, trace=False):
    nc = _get_compiled()
    q = np.ascontiguousarray(q, dtype=np.float32)
    k = np.ascontiguousarray(k, dtype=np.float32)
    v = np.ascontiguousarray(v, dtype=np.float32)
    import ml_dtypes

    in_maps = [
        {
            "qT": np.ascontiguousarray(q[i].T).astype(ml_dtypes.bfloat16),
            "kT": np.ascontiguousarray(k[i].T).astype(ml_dtypes.bfloat16),
            "v": v[i],
        }
        for i in range(N_CORES)
    ]
    res = run_bass_kernel_spmd(nc, in_maps, list(range(N_CORES)), trace=trace)
    out = np.stack([res.results[i]["out"] for i in range(N_CORES)], axis=0)
    return out.astype(np.float32, copy=False), res


def kernel(q, k, v):
    out, _ = run(q, k, v, trace=False)
    return out

